# revision 2
# baseline (speedup 1.0000x reference)
"""Trainium2 Bass kernel for the fine-grained caption/image sparse-attention
similarity module.

Math (per image i, caption-word row x = (c,w)):
    q = LN(caps;g1,b1)@Wq^T + bq          -> folded: LNraw@WqgT + qb
    k = LN(imgs;g2,b2)@Wk^T + bk          -> folded: LNraw@WkgT + bk'
    v = LN(imgs;g3,b3)@Wv^T + bv          -> folded: LNraw@WvgT + bv'
    e[x,i,r]  = exp(q.k/sqrt(D)) * mask01[i,r]        (unnormalized attn)
    ssum      = sum_r e;  a = e/ssum                  (true attn)
    ctx       = a@V_i  (rank-36!)  ->  never materialized:
      V'_i = V_i - rowmean(V_i)   (row-centered -> LN centering is exact)
      P_i  = V'_i @ (Wo*g4)^T
      S_i  = V'_i V'_i^T / D,  M_i = P_i P_i^T        (36x36 Grams)
    g   = sqrt(e^T S_i e + LN_EPS*ssum^2)             ( = ssum*sqrt(var+eps) )
    num = e.(Q@P_i^T) ;  nM = e^T M_i e
    s   = num / (sqrt(nM) + EPS*g)                    (== cos-sim numerically)
    out[i, x] = s*capm + (capm-1)                     (invalid cap words -> -1)

Sharding: 8 images per core (replicated captions/weights), no collectives.
"""

import os
import sys

import numpy as np

EPS = 1e-8
LN_EPS = 1e-5

Bi, R, Bc, W, D = 64, 36, 64, 40, 512
N_CORES = 8
BI_S = Bi // N_CORES      # 8 images per core
IR = BI_S * R             # 288
NCW = Bc * W              # 2560 caption-word rows
NT = NCW // 128           # 20 row tiles
DC = D // 128             # 4 contraction chunks
SCALE = 1.0 / float(np.sqrt(D))

for _p in ("/opt/trn_rl_repo",):
    if os.path.isdir(_p) and _p not in sys.path:
        sys.path.insert(0, _p)

LAST_EXEC_NS = None
LAST_TRACE = None
_PROG_CACHE = {}


# ----------------------------------------------------------------- fallback --
def _np_ln(x, g, b):
    mu = x.mean(axis=-1, keepdims=True, dtype=np.float32)
    xc = x - mu
    var = np.mean(xc * xc, axis=-1, keepdims=True, dtype=np.float32)
    return xc / np.sqrt(var + LN_EPS) * g + b


def _np_kernel(imgs, caps, img_lens, cap_lens, Wq, bq, Wk, bk, Wv, bv, Wo, bo,
               g1, b1, g2, b2, g3, b3, g4, b4):
    bi, r, d = imgs.shape
    bc, w, _ = caps.shape
    img_valid = np.arange(r)[None, :] < img_lens[:, None]
    cap_valid = np.arange(w)[None, :] < cap_lens[:, None]
    imgs_m = (imgs * img_valid[..., None]).astype(np.float32)
    caps_m = (caps * cap_valid[..., None]).astype(np.float32)
    q = (_np_ln(caps_m, g1, b1).reshape(bc * w, d) @ Wq.T + bq).astype(np.float32)
    lni = _np_ln(imgs_m, g2, b2).reshape(bi * r, d)
    k = (lni @ Wk.T + bk).reshape(bi, r, d).astype(np.float32)
    lni3 = _np_ln(imgs_m, g3, b3).reshape(bi * r, d)
    v = ((lni3 @ Wv.T + bv) * img_valid.reshape(bi * r, 1)).reshape(bi, r, d)
    sims = (q @ k.reshape(bi * r, d).T) * np.float32(1.0 / np.sqrt(d))
    sims = sims.reshape(bc, w, bi, r)
    pm = cap_valid[:, :, None, None] & img_valid[None, None, :, :]
    sims = np.where(pm, sims, np.float32(-1e30))
    sims -= sims.max(axis=-1, keepdims=True)
    np.exp(sims, out=sims)
    sims /= sims.sum(axis=-1, keepdims=True)
    attn = np.where(pm, sims, np.float32(0.0))
    attn_b = np.ascontiguousarray(attn.transpose(2, 0, 1, 3)).reshape(bi, bc * w, r)
    ctx = np.matmul(attn_b, v.astype(np.float32))
    out = _np_ln(ctx, g4, b4).reshape(bi * bc * w, d) @ Wo.T + bo
    out = out.reshape(bi, bc * w, d).astype(np.float32)
    num = np.einsum('bnd,nd->bn', out, q, optimize=True)
    den = np.sqrt((out * out).sum(axis=-1)) + np.float32(EPS)
    s = (num / den).reshape(bi, bc, w)
    s = np.where(cap_valid[None, :, :], s, np.float32(-1.0))
    return s.astype(np.float32)


# ------------------------------------------------------------ device program --
def _build_program():
    import concourse.bass as bass
    import concourse.tile as tile
    from concourse import mybir
    from concourse.masks import make_identity
    from contextlib import ExitStack

    dt = mybir.dt
    f32, bf16 = dt.float32, dt.bfloat16
    AF = mybir.ActivationFunctionType
    OP = mybir.AluOpType
    AX = mybir.AxisListType

    nc = bass.Bass()
    caps_d = nc.dram_tensor("caps", (NCW, D), bf16, kind="ExternalInput")
    imgs_d = nc.dram_tensor("imgs", (IR, D), bf16, kind="ExternalInput")
    wq_d = nc.dram_tensor("wq", (D, D), bf16, kind="ExternalInput")
    wk_d = nc.dram_tensor("wk", (D, D), bf16, kind="ExternalInput")
    wv_d = nc.dram_tensor("wv", (D, D), bf16, kind="ExternalInput")
    wo_d = nc.dram_tensor("wo", (D, D), bf16, kind="ExternalInput")
    bvec_d = nc.dram_tensor("bvec", (128, 16), f32, kind="ExternalInput")
    m01_d = nc.dram_tensor("mask01", (IR,), bf16, kind="ExternalInput")
    capm_d = nc.dram_tensor("capm2", (NCW, 2), f32, kind="ExternalInput")
    out_d = nc.dram_tensor("out", (BI_S, NCW), f32, kind="ExternalOutput")

    irows = [(0, 128), (128, 128), (256, 32)]

    with ExitStack() as ctx:
        tc = ctx.enter_context(tile.TileContext(nc))
        big = ctx.enter_context(tc.tile_pool(name="big", bufs=1))
        wrk = ctx.enter_context(tc.tile_pool(name="wrk", bufs=3))
        mvp = ctx.enter_context(tc.tile_pool(name="mvp", bufs=6))
        apl = ctx.enter_context(tc.tile_pool(name="apl", bufs=3))
        tpl = ctx.enter_context(tc.tile_pool(name="tpl", bufs=2))
        atp = ctx.enter_context(tc.tile_pool(name="atp", bufs=6))
        scr = ctx.enter_context(tc.tile_pool(name="scr", bufs=3))
        epp = ctx.enter_context(tc.tile_pool(name="epp", bufs=16))
        psb = ctx.enter_context(tc.tile_pool(name="psb", bufs=6, space="PSUM"))
        pst = ctx.enter_context(tc.tile_pool(name="pst", bufs=2, space="PSUM"))

        ident = big.tile([128, 128], bf16, tag="ident")
        make_identity(nc, ident)

        wsb = {}
        for nm, d_ in (("wq", wq_d), ("wk", wk_d), ("wv", wv_d), ("wo", wo_d)):
            t_ = big.tile([128, DC, D], bf16, tag=nm)
            nc.sync.dma_start(out=t_, in_=d_[:, :].rearrange("(k p) d -> p k d", p=128))
            wsb[nm] = t_
        bvec = big.tile([128, 16], f32, tag="bvec")
        nc.sync.dma_start(out=bvec, in_=bvec_d[:, :])
        mask = big.tile([128, BI_S, R], bf16, tag="mask")
        m01_ap = m01_d[:]
        m01_b = bass.AP(tensor=m01_ap.tensor, offset=m01_ap.offset,
                        ap=[[0, 128]] + list(m01_ap.ap))
        nc.gpsimd.dma_start(out=mask, in_=m01_b)
        capm = big.tile([128, NT, 2], f32, tag="capm")
        nc.sync.dma_start(out=capm, in_=capm_d[:, :].rearrange("(t p) c -> p t c", p=128))

        def layer_norm(x, p):
            """x: (p, D) tile -> bf16 row-normalized (no affine; folded)."""
            st = mvp.tile([128, 6], f32, tag="st")
            nc.vector.bn_stats(st[:p], x[:p])
            ag = mvp.tile([128, 2], f32, tag="ag")
            nc.vector.bn_aggr(ag[:p], st[:p])
            sg = mvp.tile([128, 1], f32, tag="sg")
            nc.scalar.activation(sg[:p], ag[:p, 1:2], AF.Sqrt, bias=float(LN_EPS))
            iv = mvp.tile([128, 1], f32, tag="iv")
            nc.vector.reciprocal(iv[:p], sg[:p])
            ln = wrk.tile([128, D], bf16, tag="ln")
            nc.vector.tensor_scalar(out=ln[:p], in0=x[:p],
                                    scalar1=ag[:p, 0:1], scalar2=iv[:p],
                                    op0=OP.subtract, op1=OP.mult)
            return ln

        # ---------------- phase A: image side (per-core slice, 288 rows) ----
        lniT = big.tile([128, DC, IR], bf16, tag="lniT")
        for (r0, p) in irows:
            x = wrk.tile([128, D], bf16, tag="xin")
            nc.sync.dma_start(out=x[:p], in_=imgs_d[r0:r0 + p, :])
            ln = layer_norm(x, p)
            for j in range(DC):
                pt = pst.tile([128, 128], bf16, tag="tr")
                nc.tensor.transpose(pt[:, :p], ln[:p, 128 * j:128 * j + 128],
                                    ident[:p, :p])
                nc.scalar.copy(out=lniT[:, j, r0:r0 + p], in_=pt[:, :p])

        # K^T = WkgT-proj(lniT) + bk'  (d on partitions)
        kT = big.tile([128, DC, IR], bf16, tag="kT")
        for j in range(DC):
            ps = psb.tile([128, IR], f32, tag="ps")
            for k in range(DC):
                nc.tensor.matmul(ps, lhsT=wsb["wk"][:, k, 128 * j:128 * j + 128],
                                 rhs=lniT[:, k, :], start=(k == 0), stop=(k == DC - 1))
            nc.vector.tensor_scalar_add(out=kT[:, j, :], in0=ps,
                                        scalar1=bvec[:, 4 + j:5 + j])

        # V natural -> row-center -> V'^T (+bvc per-partition)
        vpT = big.tile([128, DC, IR], bf16, tag="vpT")
        for (r0, p) in irows:
            ps = psb.tile([128, D], f32, tag="ps")
            for k in range(DC):
                nc.tensor.matmul(ps[:p], lhsT=lniT[:, k, r0:r0 + p],
                                 rhs=wsb["wv"][:, k, :], start=(k == 0), stop=(k == DC - 1))
            ms = mvp.tile([128, 1], f32, tag="ms")
            nc.vector.reduce_sum(ms[:p], ps[:p], axis=AX.X)
            mu = mvp.tile([128, 1], f32, tag="mu")
            nc.scalar.mul(mu[:p], ms[:p], 1.0 / D)
            vb = wrk.tile([128, D], bf16, tag="vb")
            nc.vector.tensor_scalar_sub(out=vb[:p], in0=ps[:p], scalar1=mu[:p])
            for j in range(DC):
                pt = pst.tile([128, 128], bf16, tag="tr")
                nc.tensor.transpose(pt[:, :p], vb[:p, 128 * j:128 * j + 128],
                                    ident[:p, :p])
                nc.vector.tensor_scalar_add(out=vpT[:, j, r0:r0 + p], in0=pt[:, :p],
                                            scalar1=bvec[:, 12 + j:13 + j])

        # P^T = Wo_g4 @ V'^T + pc
        pT = big.tile([128, DC, IR], bf16, tag="pT")
        for j in range(DC):
            ps = psb.tile([128, IR], f32, tag="ps")
            for k in range(DC):
                nc.tensor.matmul(ps, lhsT=wsb["wo"][:, k, 128 * j:128 * j + 128],
                                 rhs=vpT[:, k, :], start=(k == 0), stop=(k == DC - 1))
            nc.vector.tensor_scalar_add(out=pT[:, j, :], in0=ps,
                                        scalar1=bvec[:, 8 + j:9 + j])

        # 36x36 Grams per image: S/D and M
        sm = big.tile([36, BI_S, 72], bf16, tag="sm")
        for i in range(BI_S):
            for (src, c0_, scl) in ((vpT, 0, 1.0 / D), (pT, 36, 1.0)):
                gps = pst.tile([36, 36], f32, tag="tr")
                for k in range(DC):
                    nc.tensor.matmul(gps, lhsT=src[:, k, 36 * i:36 * i + 36],
                                     rhs=src[:, k, 36 * i:36 * i + 36],
                                     start=(k == 0), stop=(k == DC - 1))
                nc.scalar.mul(sm[:, i, c0_:c0_ + 36], gps, scl)

        # ---------------- phase B: caption LN/transpose + Q^T projection ----
        lncT = big.tile([128, DC, NCW], bf16, tag="lncT")
        qT = big.tile([128, DC, NCW], bf16, tag="qT")
        for t in range(NT):
            x = wrk.tile([128, D], bf16, tag="xin")
            nc.sync.dma_start(out=x, in_=caps_d[128 * t:128 * t + 128, :])
            ln = layer_norm(x, 128)
            for j in range(DC):
                pt = pst.tile([128, 128], bf16, tag="tr")
                nc.tensor.transpose(pt, ln[:, 128 * j:128 * j + 128], ident)
                nc.scalar.copy(out=lncT[:, j, 128 * t:128 * t + 128], in_=pt)
            if t % 4 == 3:
                c = t // 4
                for j in range(DC):
                    ps = psb.tile([128, 512], f32, tag="ps")
                    for k in range(DC):
                        nc.tensor.matmul(ps, lhsT=wsb["wq"][:, k, 128 * j:128 * j + 128],
                                         rhs=lncT[:, k, 512 * c:512 * c + 512],
                                         start=(k == 0), stop=(k == DC - 1))
                    nc.scalar.activation(qT[:, j, 512 * c:512 * c + 512], ps,
                                         AF.Identity, bias=bvec[:, j:j + 1])

        # ---------------- phase C: per caption-word tile ---------------------
        for t in range(NT):
            cw = slice(128 * t, 128 * t + 128)
            psK = psb.tile([128, BI_S, R], f32, tag="ps")
            for k in range(DC):
                nc.tensor.matmul(psK, lhsT=qT[:, k, cw], rhs=kT[:, k, :],
                                 start=(k == 0), stop=(k == DC - 1))
            psT = psb.tile([128, BI_S, R], f32, tag="ps")
            for k in range(DC):
                nc.tensor.matmul(psT, lhsT=qT[:, k, cw], rhs=pT[:, k, :],
                                 start=(k == 0), stop=(k == DC - 1))
            A = apl.tile([128, BI_S, R], bf16, tag="A")
            nc.scalar.activation(A, psK, AF.Exp, scale=SCALE)
            nc.vector.tensor_mul(A, A, mask)
            tsb = tpl.tile([128, BI_S, R], bf16, tag="tsb")
            nc.scalar.copy(tsb, psT)

            ats = []
            for g3 in range(3):
                ni = 3 if g3 < 2 else 2
                wdt = 36 * ni
                pt = pst.tile([128, 128], bf16, tag="tr")
                nc.tensor.transpose(pt[:wdt, :], A[:, 3 * g3:3 * g3 + ni, :], ident)
                at = atp.tile([128, 128], bf16, tag="at")
                nc.scalar.copy(at[:wdt, :], pt[:wdt, :])
                ats.append(at)

            psB = []
            for h in range(2):
                pb = psb.tile([128, 4, 72], f32, tag="ps")
                for m in range(4):
                    i = 4 * h + m
                    nc.tensor.matmul(pb[:, m, :],
                                     lhsT=ats[i // 3][36 * (i % 3):36 * (i % 3) + 36, :],
                                     rhs=sm[:, i, :], start=True, stop=True)
                psB.append(pb)

            ssum = epp.tile([128, BI_S], f32, tag="essum")
            nc.vector.reduce_sum(ssum, A, axis=AX.X)
            scT = scr.tile([128, BI_S, R], bf16, tag="scT")
            nc.vector.tensor_mul(scT, A, tsb)
            eT = epp.tile([128, BI_S], f32, tag="enT")
            nc.vector.reduce_sum(eT, scT, axis=AX.X)
            scS = scr.tile([128, BI_S, R], f32, tag="scS")
            scM = scr.tile([128, BI_S, R], f32, tag="scM")
            for h in range(2):
                sl = slice(4 * h, 4 * h + 4)
                nc.vector.tensor_mul(scS[:, sl, :], A[:, sl, :], psB[h][:, :, 0:36])
                nc.vector.tensor_mul(scM[:, sl, :], A[:, sl, :], psB[h][:, :, 36:72])
            eS = epp.tile([128, BI_S], f32, tag="enS")
            nc.vector.reduce_sum(eS, scS, axis=AX.X)
            eM = epp.tile([128, BI_S], f32, tag="enM")
            nc.vector.reduce_sum(eM, scM, axis=AX.X)

            sqe = epp.tile([128, BI_S], f32, tag="esqe")
            nc.scalar.activation(sqe, ssum, AF.Square, scale=float(np.sqrt(LN_EPS)))
            t2 = epp.tile([128, BI_S], f32, tag="et2")
            nc.gpsimd.tensor_add(t2, sqe, eS)
            gg = epp.tile([128, BI_S], f32, tag="egg")
            nc.scalar.activation(gg, t2, AF.Sqrt)
            qm = epp.tile([128, BI_S], f32, tag="eqm")
            nc.scalar.activation(qm, eM, AF.Sqrt)
            ge = epp.tile([128, BI_S], f32, tag="ege")
            nc.gpsimd.tensor_scalar_mul(ge, gg, float(EPS))
            den = epp.tile([128, BI_S], f32, tag="eden")
            nc.gpsimd.tensor_add(den, ge, qm)
            rr = epp.tile([128, BI_S], f32, tag="err")
            nc.vector.reciprocal(rr, den)
            s0 = epp.tile([128, BI_S], f32, tag="es0")
            nc.gpsimd.tensor_mul(s0, eT, rr)
            sf = epp.tile([128, BI_S], f32, tag="esf")
            nc.gpsimd.tensor_scalar(out=sf, in0=s0,
                                    scalar1=capm[:, t, 0:1], scalar2=capm[:, t, 1:2],
                                    op0=OP.mult, op1=OP.add)
            nc.sync.dma_start(out=out_d[:, cw].rearrange("i p -> p i"), in_=sf)

    return nc


def _get_program():
    if "nc" not in _PROG_CACHE:
        _PROG_CACHE["nc"] = _build_program()
    return _PROG_CACHE["nc"]


# ------------------------------------------------------------------- driver --
def kernel(imgs, caps, img_lens, cap_lens,
           Wq, bq, Wk, bk, Wv, bv, Wo, bo,
           g1, b1, g2, b2, g3, b3, g4, b4):
    global LAST_EXEC_NS, LAST_TRACE
    args = dict(imgs=imgs, caps=caps, img_lens=img_lens, cap_lens=cap_lens,
                Wq=Wq, bq=bq, Wk=Wk, bk=bk, Wv=Wv, bv=bv, Wo=Wo, bo=bo,
                g1=g1, b1=b1, g2=g2, b2=b2, g3=g3, b3=b3, g4=g4, b4=b4)
    args = {k: np.asarray(v, np.float32) if np.asarray(v).dtype != np.int32
            else np.asarray(v) for k, v in args.items()}
    imgs, caps = args["imgs"], args["caps"]
    img_lens, cap_lens = np.asarray(img_lens, np.int32), np.asarray(cap_lens, np.int32)
    c0 = args["Wo"] @ args["b4"] + args["bo"]
    if (imgs.shape, caps.shape) != ((Bi, R, D), (Bc, W, D)) or np.abs(c0).max() != 0:
        return _np_kernel(**args)
    try:
        return _device_kernel(args, img_lens, cap_lens)
    except Exception:
        import traceback
        traceback.print_exc()
        print("kernel: device path failed; falling back to numpy", file=sys.stderr)
        return _np_kernel(**args)


def _device_kernel(a, img_lens, cap_lens):
    global LAST_EXEC_NS, LAST_TRACE
    import ml_dtypes
    from concourse.bass_utils import run_bass_kernel_spmd

    bf = ml_dtypes.bfloat16
    img_valid = (np.arange(R)[None, :] < img_lens[:, None])
    cap_valid = (np.arange(W)[None, :] < cap_lens[:, None])
    imgs_m = (a["imgs"] * img_valid[..., None]).reshape(Bi * R, D)
    caps_m = (a["caps"] * cap_valid[..., None]).reshape(NCW, D)

    WqgT = np.ascontiguousarray((a["Wq"] * a["g1"][None, :]).T).astype(bf)
    WkgT = np.ascontiguousarray((a["Wk"] * a["g2"][None, :]).T).astype(bf)
    WvgT = np.ascontiguousarray((a["Wv"] * a["g3"][None, :]).T).astype(bf)
    WogT = np.ascontiguousarray((a["Wo"] * a["g4"][None, :]).T).astype(bf)
    qb = a["Wq"] @ a["b1"] + a["bq"]
    bk_ = a["Wk"] @ a["b2"] + a["bk"]
    bv_ = a["Wv"] @ a["b3"] + a["bv"]
    bvc = (bv_ - bv_.mean()).astype(np.float32)
    pc = (a["Wo"] * a["g4"][None, :]) @ bvc
    bvec = np.stack([qb.reshape(DC, 128), bk_.reshape(DC, 128),
                     pc.reshape(DC, 128), bvc.reshape(DC, 128)],
                    axis=0).reshape(16, 128).T
    bvec = np.ascontiguousarray(bvec, dtype=np.float32)  # (128, 16)

    capm = cap_valid.reshape(NCW, 1).astype(np.float32)
    capm2 = np.ascontiguousarray(np.concatenate([capm, capm - 1.0], axis=1))

    caps_bf = np.ascontiguousarray(caps_m).astype(bf)
    in_maps = []
    for c in range(N_CORES):
        sl = slice(c * BI_S * R, (c + 1) * BI_S * R)
        in_maps.append({
            "caps": caps_bf,
            "imgs": np.ascontiguousarray(imgs_m[sl]).astype(bf),
            "wq": WqgT, "wk": WkgT, "wv": WvgT, "wo": WogT,
            "bvec": bvec,
            "mask01": np.ascontiguousarray(
                img_valid[c * BI_S:(c + 1) * BI_S].reshape(IR)).astype(bf),
            "capm2": capm2,
        })

    nc = _get_program()
    trace = bool(os.environ.get("BASS_KTRACE"))
    kw = {}
    if trace:
        kw = dict(trace=True, tmpdir=os.environ.get("BASS_KTRACE_DIR") or None)
    res = run_bass_kernel_spmd(nc, in_maps, list(range(N_CORES)), **kw)
    if trace:
        LAST_EXEC_NS = res.exec_time_ns
        LAST_TRACE = res.profile_json
    out = np.concatenate(
        [r["out"].reshape(BI_S, Bc, W) for r in res.results], axis=0)
    return np.ascontiguousarray(out.astype(np.float32))


# revision 4
# speedup vs baseline: 1.0402x; 1.0402x over previous
"""Trainium2 Bass kernel for the fine-grained caption/image sparse-attention
similarity module.

Math (per image i, caption-word row x = (c,w)):
    q = LN(caps;g1,b1)@Wq^T + bq          -> folded: LNraw@WqgT + qb
    k = LN(imgs;g2,b2)@Wk^T + bk          -> folded: LNraw@WkgT + bk'
    v = LN(imgs;g3,b3)@Wv^T + bv          -> folded: LNraw@WvgT + bv'
    e[x,i,r]  = exp(q.k/sqrt(D)) * mask01[i,r]        (unnormalized attn)
    ssum      = sum_r e;  a = e/ssum                  (true attn)
    ctx       = a@V_i  (rank-36!)  ->  never materialized:
      V'_i = V_i - rowmean(V_i)   (row-centered -> LN centering is exact)
      P_i  = V'_i @ (Wo*g4)^T
      S_i  = V'_i V'_i^T / D,  M_i = P_i P_i^T        (36x36 Grams)
    g   = sqrt(e^T S_i e + LN_EPS*ssum^2)             ( = ssum*sqrt(var+eps) )
    num = e.(Q@P_i^T) ;  nM = e^T M_i e
    s   = num / (sqrt(nM) + EPS*g)                    (== cos-sim numerically)
    out[i, x] = s*capm + (capm-1)                     (invalid cap words -> -1)

Sharding: 8 images per core (replicated captions/weights), no collectives.
"""

import os
import sys

import numpy as np

EPS = 1e-8
LN_EPS = 1e-5

Bi, R, Bc, W, D = 64, 36, 64, 40, 512
N_CORES = 8
BI_S = Bi // N_CORES      # 8 images per core
IR = BI_S * R             # 288
NCW = Bc * W              # 2560 caption-word rows
NT = NCW // 128           # 20 row tiles
DC = D // 128             # 4 contraction chunks
SCALE = 1.0 / float(np.sqrt(D))

for _p in ("/opt/trn_rl_repo",):
    if os.path.isdir(_p) and _p not in sys.path:
        sys.path.insert(0, _p)

LAST_EXEC_NS = None
LAST_TRACE = None
_PROG_CACHE = {}


# ----------------------------------------------------------------- fallback --
def _np_ln(x, g, b):
    mu = x.mean(axis=-1, keepdims=True, dtype=np.float32)
    xc = x - mu
    var = np.mean(xc * xc, axis=-1, keepdims=True, dtype=np.float32)
    return xc / np.sqrt(var + LN_EPS) * g + b


def _np_kernel(imgs, caps, img_lens, cap_lens, Wq, bq, Wk, bk, Wv, bv, Wo, bo,
               g1, b1, g2, b2, g3, b3, g4, b4):
    bi, r, d = imgs.shape
    bc, w, _ = caps.shape
    img_valid = np.arange(r)[None, :] < img_lens[:, None]
    cap_valid = np.arange(w)[None, :] < cap_lens[:, None]
    imgs_m = (imgs * img_valid[..., None]).astype(np.float32)
    caps_m = (caps * cap_valid[..., None]).astype(np.float32)
    q = (_np_ln(caps_m, g1, b1).reshape(bc * w, d) @ Wq.T + bq).astype(np.float32)
    lni = _np_ln(imgs_m, g2, b2).reshape(bi * r, d)
    k = (lni @ Wk.T + bk).reshape(bi, r, d).astype(np.float32)
    lni3 = _np_ln(imgs_m, g3, b3).reshape(bi * r, d)
    v = ((lni3 @ Wv.T + bv) * img_valid.reshape(bi * r, 1)).reshape(bi, r, d)
    sims = (q @ k.reshape(bi * r, d).T) * np.float32(1.0 / np.sqrt(d))
    sims = sims.reshape(bc, w, bi, r)
    pm = cap_valid[:, :, None, None] & img_valid[None, None, :, :]
    sims = np.where(pm, sims, np.float32(-1e30))
    sims -= sims.max(axis=-1, keepdims=True)
    np.exp(sims, out=sims)
    sims /= sims.sum(axis=-1, keepdims=True)
    attn = np.where(pm, sims, np.float32(0.0))
    attn_b = np.ascontiguousarray(attn.transpose(2, 0, 1, 3)).reshape(bi, bc * w, r)
    ctx = np.matmul(attn_b, v.astype(np.float32))
    out = _np_ln(ctx, g4, b4).reshape(bi * bc * w, d) @ Wo.T + bo
    out = out.reshape(bi, bc * w, d).astype(np.float32)
    num = np.einsum('bnd,nd->bn', out, q, optimize=True)
    den = np.sqrt((out * out).sum(axis=-1)) + np.float32(EPS)
    s = (num / den).reshape(bi, bc, w)
    s = np.where(cap_valid[None, :, :], s, np.float32(-1.0))
    return s.astype(np.float32)


# ------------------------------------------------------------ device program --
def _build_program():
    import concourse.bass as bass
    import concourse.tile as tile
    from concourse import mybir
    from concourse.masks import make_identity
    from contextlib import ExitStack

    dt = mybir.dt
    f32, bf16 = dt.float32, dt.bfloat16
    AF = mybir.ActivationFunctionType
    OP = mybir.AluOpType
    AX = mybir.AxisListType

    nc = bass.Bass()
    caps_d = nc.dram_tensor("caps", (NCW, D), bf16, kind="ExternalInput")
    imgs_d = nc.dram_tensor("imgs", (IR, D), bf16, kind="ExternalInput")
    wq_d = nc.dram_tensor("wq", (D, D), bf16, kind="ExternalInput")
    wk_d = nc.dram_tensor("wk", (D, D), bf16, kind="ExternalInput")
    wv_d = nc.dram_tensor("wv", (D, D), bf16, kind="ExternalInput")
    wo_d = nc.dram_tensor("wo", (D, D), bf16, kind="ExternalInput")
    bvec_d = nc.dram_tensor("bvec", (128, 16), f32, kind="ExternalInput")
    m01_d = nc.dram_tensor("mask01", (IR,), bf16, kind="ExternalInput")
    capm_d = nc.dram_tensor("capm2", (NCW, 2), f32, kind="ExternalInput")
    out_d = nc.dram_tensor("out", (BI_S, NCW), f32, kind="ExternalOutput")

    irows = [(0, 128), (128, 128), (256, 32)]

    with ExitStack() as ctx:
        tc = ctx.enter_context(tile.TileContext(nc))
        big = ctx.enter_context(tc.tile_pool(name="big", bufs=1))
        wrk = ctx.enter_context(tc.tile_pool(name="wrk", bufs=3))
        mvp = ctx.enter_context(tc.tile_pool(name="mvp", bufs=6))
        apl = ctx.enter_context(tc.tile_pool(name="apl", bufs=3))
        tpl = ctx.enter_context(tc.tile_pool(name="tpl", bufs=2))
        atp = ctx.enter_context(tc.tile_pool(name="atp", bufs=6))
        scr = ctx.enter_context(tc.tile_pool(name="scr", bufs=3))
        epp = ctx.enter_context(tc.tile_pool(name="epp", bufs=16))
        psb = ctx.enter_context(tc.tile_pool(name="psb", bufs=6, space="PSUM"))
        pst = ctx.enter_context(tc.tile_pool(name="pst", bufs=2, space="PSUM"))

        ident = big.tile([128, 128], bf16, tag="ident")
        make_identity(nc, ident)
        epsb = big.tile([128, 1], f32, tag="epsb")
        nc.vector.memset(epsb, float(LN_EPS))

        wsb = {}
        for nm, d_ in (("wq", wq_d), ("wk", wk_d), ("wv", wv_d), ("wo", wo_d)):
            t_ = big.tile([128, DC, D], bf16, tag=nm)
            nc.sync.dma_start(out=t_, in_=d_[:, :].rearrange("(k p) d -> p k d", p=128))
            wsb[nm] = t_
        bvec = big.tile([128, 16], f32, tag="bvec")
        nc.sync.dma_start(out=bvec, in_=bvec_d[:, :])
        mask = big.tile([128, BI_S, R], bf16, tag="mask")
        m01_ap = m01_d[:]
        m01_b = bass.AP(tensor=m01_ap.tensor, offset=m01_ap.offset,
                        ap=[[0, 128]] + list(m01_ap.ap))
        nc.gpsimd.dma_start(out=mask, in_=m01_b)
        capm = big.tile([128, NT, 2], f32, tag="capm")
        nc.sync.dma_start(out=capm, in_=capm_d[:, :].rearrange("(t p) c -> p t c", p=128))

        def layer_norm(x, p):
            """x: (p, D) tile -> bf16 row-normalized (no affine; folded)."""
            st = mvp.tile([128, 6], f32, tag="st")
            nc.vector.bn_stats(st[:p], x[:p])
            ag = mvp.tile([128, 2], f32, tag="ag")
            nc.vector.bn_aggr(ag[:p], st[:p])
            sg = mvp.tile([128, 1], f32, tag="sg")
            nc.scalar.activation(sg[:p], ag[:p, 1:2], AF.Sqrt, bias=epsb[:p])
            iv = mvp.tile([128, 1], f32, tag="iv")
            nc.vector.reciprocal(iv[:p], sg[:p])
            ln = wrk.tile([128, D], bf16, tag="ln")
            nc.vector.tensor_scalar(out=ln[:p], in0=x[:p],
                                    scalar1=ag[:p, 0:1], scalar2=iv[:p],
                                    op0=OP.subtract, op1=OP.mult)
            return ln

        # ---------------- phase A: image side (per-core slice, 288 rows) ----
        lniT = big.tile([128, DC, IR], bf16, tag="lniT")
        for (r0, p) in irows:
            x = wrk.tile([128, D], bf16, tag="xin")
            nc.sync.dma_start(out=x[:p], in_=imgs_d[r0:r0 + p, :])
            ln = layer_norm(x, p)
            for j in range(DC):
                pt = pst.tile([128, 128], bf16, tag="tr")
                nc.tensor.transpose(pt[:, :p], ln[:p, 128 * j:128 * j + 128],
                                    ident[:p, :p])
                nc.scalar.copy(out=lniT[:, j, r0:r0 + p], in_=pt[:, :p])

        # K^T = WkgT-proj(lniT) + bk'  (d on partitions)
        kT = big.tile([128, DC, IR], bf16, tag="kT")
        for j in range(DC):
            ps = psb.tile([128, IR], f32, tag="ps")
            for k in range(DC):
                nc.tensor.matmul(ps, lhsT=wsb["wk"][:, k, 128 * j:128 * j + 128],
                                 rhs=lniT[:, k, :], start=(k == 0), stop=(k == DC - 1))
            nc.vector.tensor_scalar_add(out=kT[:, j, :], in0=ps,
                                        scalar1=bvec[:, 4 + j:5 + j])

        # V natural -> row-center -> V'^T (+bvc per-partition)
        vpT = big.tile([128, DC, IR], bf16, tag="vpT")
        for (r0, p) in irows:
            ps = psb.tile([128, D], f32, tag="ps")
            for k in range(DC):
                nc.tensor.matmul(ps[:p], lhsT=lniT[:, k, r0:r0 + p],
                                 rhs=wsb["wv"][:, k, :], start=(k == 0), stop=(k == DC - 1))
            ms = mvp.tile([128, 1], f32, tag="ms")
            nc.vector.reduce_sum(ms[:p], ps[:p], axis=AX.X)
            mu = mvp.tile([128, 1], f32, tag="mu")
            nc.scalar.mul(mu[:p], ms[:p], 1.0 / D)
            vb = wrk.tile([128, D], bf16, tag="vb")
            nc.vector.tensor_scalar_sub(out=vb[:p], in0=ps[:p], scalar1=mu[:p])
            for j in range(DC):
                pt = pst.tile([128, 128], bf16, tag="tr")
                nc.tensor.transpose(pt[:, :p], vb[:p, 128 * j:128 * j + 128],
                                    ident[:p, :p])
                nc.vector.tensor_scalar_add(out=vpT[:, j, r0:r0 + p], in0=pt[:, :p],
                                            scalar1=bvec[:, 12 + j:13 + j])

        # P^T = Wo_g4 @ V'^T + pc
        pT = big.tile([128, DC, IR], bf16, tag="pT")
        for j in range(DC):
            ps = psb.tile([128, IR], f32, tag="ps")
            for k in range(DC):
                nc.tensor.matmul(ps, lhsT=wsb["wo"][:, k, 128 * j:128 * j + 128],
                                 rhs=vpT[:, k, :], start=(k == 0), stop=(k == DC - 1))
            nc.vector.tensor_scalar_add(out=pT[:, j, :], in0=ps,
                                        scalar1=bvec[:, 8 + j:9 + j])

        # 36x36 Grams per image: S/D and M
        sm = big.tile([36, BI_S, 72], bf16, tag="sm")
        for i in range(BI_S):
            for (src, c0_, scl) in ((vpT, 0, 1.0 / D), (pT, 36, 1.0)):
                gps = pst.tile([36, 36], f32, tag="tr")
                for k in range(DC):
                    nc.tensor.matmul(gps, lhsT=src[:, k, 36 * i:36 * i + 36],
                                     rhs=src[:, k, 36 * i:36 * i + 36],
                                     start=(k == 0), stop=(k == DC - 1))
                nc.scalar.mul(sm[:, i, c0_:c0_ + 36], gps, scl)

        # ---------------- phase B: caption LN/transpose + Q^T projection ----
        lncT = big.tile([128, DC, NCW], bf16, tag="lncT")
        qT = big.tile([128, DC, NCW], bf16, tag="qT")
        for t in range(NT):
            x = wrk.tile([128, D], bf16, tag="xin")
            nc.sync.dma_start(out=x, in_=caps_d[128 * t:128 * t + 128, :])
            ln = layer_norm(x, 128)
            for j in range(DC):
                pt = pst.tile([128, 128], bf16, tag="tr")
                nc.tensor.transpose(pt, ln[:, 128 * j:128 * j + 128], ident)
                nc.scalar.copy(out=lncT[:, j, 128 * t:128 * t + 128], in_=pt)
            if t % 4 == 3:
                c = t // 4
                for j in range(DC):
                    ps = psb.tile([128, 512], f32, tag="ps")
                    for k in range(DC):
                        nc.tensor.matmul(ps, lhsT=wsb["wq"][:, k, 128 * j:128 * j + 128],
                                         rhs=lncT[:, k, 512 * c:512 * c + 512],
                                         start=(k == 0), stop=(k == DC - 1))
                    nc.scalar.activation(qT[:, j, 512 * c:512 * c + 512], ps,
                                         AF.Identity, bias=bvec[:, j:j + 1])

        # ---------------- phase C: per caption-word tile ---------------------
        for t in range(NT):
            cw = slice(128 * t, 128 * t + 128)
            psK = psb.tile([128, BI_S, R], f32, tag="ps")
            for k in range(DC):
                nc.tensor.matmul(psK, lhsT=qT[:, k, cw], rhs=kT[:, k, :],
                                 start=(k == 0), stop=(k == DC - 1))
            psT = psb.tile([128, BI_S, R], f32, tag="ps")
            for k in range(DC):
                nc.tensor.matmul(psT, lhsT=qT[:, k, cw], rhs=pT[:, k, :],
                                 start=(k == 0), stop=(k == DC - 1))
            A = apl.tile([128, BI_S, R], bf16, tag="A")
            nc.scalar.activation(A, psK, AF.Exp, scale=SCALE)
            nc.vector.tensor_mul(A, A, mask)
            tsb = tpl.tile([128, BI_S, R], bf16, tag="tsb")
            nc.scalar.copy(tsb, psT)

            ats = []
            for g3 in range(3):
                ni = 3 if g3 < 2 else 2
                wdt = 36 * ni
                pt = pst.tile([128, 128], bf16, tag="tr")
                nc.tensor.transpose(pt[:wdt, :], A[:, 3 * g3:3 * g3 + ni, :], ident)
                at = atp.tile([128, 128], bf16, tag="at")
                nc.scalar.copy(at[:wdt, :], pt[:wdt, :])
                ats.append(at)

            psB = []
            for h in range(2):
                pb = psb.tile([128, 4, 72], f32, tag="ps")
                for m in range(4):
                    i = 4 * h + m
                    nc.tensor.matmul(pb[:, m, :],
                                     lhsT=ats[i // 3][36 * (i % 3):36 * (i % 3) + 36, :],
                                     rhs=sm[:, i, :], start=True, stop=True)
                psB.append(pb)

            ssum = epp.tile([128, BI_S], f32, tag="essum")
            nc.vector.reduce_sum(ssum, A, axis=AX.X)
            scT = scr.tile([128, BI_S, R], bf16, tag="scT")
            nc.vector.tensor_mul(scT, A, tsb)
            eT = epp.tile([128, BI_S], f32, tag="enT")
            nc.vector.reduce_sum(eT, scT, axis=AX.X)
            scS = scr.tile([128, BI_S, R], f32, tag="scS")
            scM = scr.tile([128, BI_S, R], f32, tag="scM")
            for h in range(2):
                sl = slice(4 * h, 4 * h + 4)
                nc.vector.tensor_mul(scS[:, sl, :], A[:, sl, :], psB[h][:, :, 0:36])
                nc.vector.tensor_mul(scM[:, sl, :], A[:, sl, :], psB[h][:, :, 36:72])
            eS = epp.tile([128, BI_S], f32, tag="enS")
            nc.vector.reduce_sum(eS, scS, axis=AX.X)
            eM = epp.tile([128, BI_S], f32, tag="enM")
            nc.vector.reduce_sum(eM, scM, axis=AX.X)

            sqe = epp.tile([128, BI_S], f32, tag="esqe")
            nc.scalar.activation(sqe, ssum, AF.Square, scale=float(np.sqrt(LN_EPS)))
            t2 = epp.tile([128, BI_S], f32, tag="et2")
            nc.gpsimd.tensor_add(t2, sqe, eS)
            gg = epp.tile([128, BI_S], f32, tag="egg")
            nc.scalar.activation(gg, t2, AF.Sqrt)
            qm = epp.tile([128, BI_S], f32, tag="eqm")
            nc.scalar.activation(qm, eM, AF.Sqrt)
            ge = epp.tile([128, BI_S], f32, tag="ege")
            nc.gpsimd.tensor_scalar_mul(ge, gg, float(EPS))
            den = epp.tile([128, BI_S], f32, tag="eden")
            nc.gpsimd.tensor_add(den, ge, qm)
            rr = epp.tile([128, BI_S], f32, tag="err")
            nc.vector.reciprocal(rr, den)
            s0 = epp.tile([128, BI_S], f32, tag="es0")
            nc.gpsimd.tensor_mul(s0, eT, rr)
            sf = epp.tile([128, BI_S], f32, tag="esf")
            nc.gpsimd.tensor_scalar(out=sf, in0=s0,
                                    scalar1=capm[:, t, 0:1], scalar2=capm[:, t, 1:2],
                                    op0=OP.mult, op1=OP.add)
            nc.sync.dma_start(out=out_d[:, cw].rearrange("i p -> p i"), in_=sf)

    return nc


def _get_program():
    if "nc" not in _PROG_CACHE:
        _PROG_CACHE["nc"] = _build_program()
    return _PROG_CACHE["nc"]


# ------------------------------------------------------------------- driver --
def kernel(imgs, caps, img_lens, cap_lens,
           Wq, bq, Wk, bk, Wv, bv, Wo, bo,
           g1, b1, g2, b2, g3, b3, g4, b4):
    global LAST_EXEC_NS, LAST_TRACE
    args = dict(imgs=imgs, caps=caps, img_lens=img_lens, cap_lens=cap_lens,
                Wq=Wq, bq=bq, Wk=Wk, bk=bk, Wv=Wv, bv=bv, Wo=Wo, bo=bo,
                g1=g1, b1=b1, g2=g2, b2=b2, g3=g3, b3=b3, g4=g4, b4=b4)
    args = {k: np.asarray(v, np.float32) if np.asarray(v).dtype != np.int32
            else np.asarray(v) for k, v in args.items()}
    imgs, caps = args["imgs"], args["caps"]
    img_lens, cap_lens = np.asarray(img_lens, np.int32), np.asarray(cap_lens, np.int32)
    c0 = args["Wo"] @ args["b4"] + args["bo"]
    if (imgs.shape, caps.shape) != ((Bi, R, D), (Bc, W, D)) or np.abs(c0).max() != 0:
        return _np_kernel(**args)
    try:
        return _device_kernel(args, img_lens, cap_lens)
    except Exception:
        import traceback
        traceback.print_exc()
        print("kernel: device path failed; falling back to numpy", file=sys.stderr)
        return _np_kernel(**args)


def _device_kernel(a, img_lens, cap_lens):
    global LAST_EXEC_NS, LAST_TRACE
    import ml_dtypes
    from concourse.bass_utils import run_bass_kernel_spmd

    bf = ml_dtypes.bfloat16
    img_valid = (np.arange(R)[None, :] < img_lens[:, None])
    cap_valid = (np.arange(W)[None, :] < cap_lens[:, None])
    imgs_m = (a["imgs"] * img_valid[..., None]).reshape(Bi * R, D)
    caps_m = (a["caps"] * cap_valid[..., None]).reshape(NCW, D)

    WqgT = np.ascontiguousarray((a["Wq"] * a["g1"][None, :]).T).astype(bf)
    WkgT = np.ascontiguousarray((a["Wk"] * a["g2"][None, :]).T).astype(bf)
    WvgT = np.ascontiguousarray((a["Wv"] * a["g3"][None, :]).T).astype(bf)
    WogT = np.ascontiguousarray((a["Wo"] * a["g4"][None, :]).T).astype(bf)
    qb = a["Wq"] @ a["b1"] + a["bq"]
    bk_ = a["Wk"] @ a["b2"] + a["bk"]
    bv_ = a["Wv"] @ a["b3"] + a["bv"]
    bvc = (bv_ - bv_.mean()).astype(np.float32)
    pc = (a["Wo"] * a["g4"][None, :]) @ bvc
    bvec = np.stack([qb.reshape(DC, 128), bk_.reshape(DC, 128),
                     pc.reshape(DC, 128), bvc.reshape(DC, 128)],
                    axis=0).reshape(16, 128).T
    bvec = np.ascontiguousarray(bvec, dtype=np.float32)  # (128, 16)

    capm = cap_valid.reshape(NCW, 1).astype(np.float32)
    capm2 = np.ascontiguousarray(np.concatenate([capm, capm - 1.0], axis=1))

    caps_bf = np.ascontiguousarray(caps_m).astype(bf)
    in_maps = []
    for c in range(N_CORES):
        sl = slice(c * BI_S * R, (c + 1) * BI_S * R)
        in_maps.append({
            "caps": caps_bf,
            "imgs": np.ascontiguousarray(imgs_m[sl]).astype(bf),
            "wq": WqgT, "wk": WkgT, "wv": WvgT, "wo": WogT,
            "bvec": bvec,
            "mask01": np.ascontiguousarray(
                img_valid[c * BI_S:(c + 1) * BI_S].reshape(IR)).astype(bf),
            "capm2": capm2,
        })

    nc = _get_program()
    trace = bool(os.environ.get("BASS_KTRACE"))
    kw = {}
    if trace:
        kw = dict(trace=True, tmpdir=os.environ.get("BASS_KTRACE_DIR") or None)
    res = run_bass_kernel_spmd(nc, in_maps, list(range(N_CORES)), **kw)
    if trace:
        LAST_EXEC_NS = res.exec_time_ns
        LAST_TRACE = res.profile_json
    out = np.concatenate(
        [r["out"].reshape(BI_S, Bc, W) for r in res.results], axis=0)
    return np.ascontiguousarray(out.astype(np.float32))


# revision 16
# speedup vs baseline: 1.0602x; 1.0192x over previous
"""Trainium2 Bass kernel for the fine-grained caption/image sparse-attention
similarity module.

Math (per image i, caption-word row x = (c,w)):
    q = LN(caps;g1,b1)@Wq^T + bq          -> folded: LNraw@WqgT + qb
    k = LN(imgs;g2,b2)@Wk^T + bk          -> folded: LNraw@WkgT + bk'
    v = LN(imgs;g3,b3)@Wv^T + bv          -> folded: LNraw@WvgT + bv'
    e[x,i,r]  = exp(q.k/sqrt(D)) * mask01[i,r]        (unnormalized attn)
    ssum      = sum_r e;  a = e/ssum                  (true attn)
    ctx       = a@V_i  (rank-36!)  ->  never materialized:
      V'_i = V_i - rowmean(V_i)   (row-centered -> LN centering is exact)
      P_i  = V'_i @ (Wo*g4)^T
      S_i  = V'_i V'_i^T / D,  M_i = P_i P_i^T        (36x36 Grams)
    g   = sqrt(e^T S_i e + LN_EPS*ssum^2)             ( = ssum*sqrt(var+eps) )
    num = e.(Q@P_i^T) ;  nM = e^T M_i e
    s   = num / (sqrt(nM) + EPS*g)                    (== cos-sim numerically)
    out[i, x] = s*capm + (capm-1)                     (invalid cap words -> -1)

Sharding: 8 images per core (replicated captions/weights), no collectives.
"""

import os
import sys

import numpy as np

EPS = 1e-8
LN_EPS = 1e-5

Bi, R, Bc, W, D = 64, 36, 64, 40, 512
N_CORES = 8
BI_S = Bi // N_CORES      # 8 images per core
IR = BI_S * R             # 288
NCW = Bc * W              # 2560 caption-word rows
NT = NCW // 128           # 20 row tiles
DC = D // 128             # 4 contraction chunks
SCALE = 1.0 / float(np.sqrt(D))

for _p in ("/opt/trn_rl_repo",):
    if os.path.isdir(_p) and _p not in sys.path:
        sys.path.insert(0, _p)

LAST_EXEC_NS = None
LAST_TRACE = None
_PROG_CACHE = {}


# ----------------------------------------------------------------- fallback --
def _np_ln(x, g, b):
    mu = x.mean(axis=-1, keepdims=True, dtype=np.float32)
    xc = x - mu
    var = np.mean(xc * xc, axis=-1, keepdims=True, dtype=np.float32)
    return xc / np.sqrt(var + LN_EPS) * g + b


def _np_kernel(imgs, caps, img_lens, cap_lens, Wq, bq, Wk, bk, Wv, bv, Wo, bo,
               g1, b1, g2, b2, g3, b3, g4, b4):
    bi, r, d = imgs.shape
    bc, w, _ = caps.shape
    img_valid = np.arange(r)[None, :] < img_lens[:, None]
    cap_valid = np.arange(w)[None, :] < cap_lens[:, None]
    imgs_m = (imgs * img_valid[..., None]).astype(np.float32)
    caps_m = (caps * cap_valid[..., None]).astype(np.float32)
    q = (_np_ln(caps_m, g1, b1).reshape(bc * w, d) @ Wq.T + bq).astype(np.float32)
    lni = _np_ln(imgs_m, g2, b2).reshape(bi * r, d)
    k = (lni @ Wk.T + bk).reshape(bi, r, d).astype(np.float32)
    lni3 = _np_ln(imgs_m, g3, b3).reshape(bi * r, d)
    v = ((lni3 @ Wv.T + bv) * img_valid.reshape(bi * r, 1)).reshape(bi, r, d)
    sims = (q @ k.reshape(bi * r, d).T) * np.float32(1.0 / np.sqrt(d))
    sims = sims.reshape(bc, w, bi, r)
    pm = cap_valid[:, :, None, None] & img_valid[None, None, :, :]
    sims = np.where(pm, sims, np.float32(-1e30))
    sims -= sims.max(axis=-1, keepdims=True)
    np.exp(sims, out=sims)
    sims /= sims.sum(axis=-1, keepdims=True)
    attn = np.where(pm, sims, np.float32(0.0))
    attn_b = np.ascontiguousarray(attn.transpose(2, 0, 1, 3)).reshape(bi, bc * w, r)
    ctx = np.matmul(attn_b, v.astype(np.float32))
    out = _np_ln(ctx, g4, b4).reshape(bi * bc * w, d) @ Wo.T + bo
    out = out.reshape(bi, bc * w, d).astype(np.float32)
    num = np.einsum('bnd,nd->bn', out, q, optimize=True)
    den = np.sqrt((out * out).sum(axis=-1)) + np.float32(EPS)
    s = (num / den).reshape(bi, bc, w)
    s = np.where(cap_valid[None, :, :], s, np.float32(-1.0))
    return s.astype(np.float32)


# ------------------------------------------------------------ device program --
def _build_program():
    import concourse.bass as bass
    import concourse.tile as tile
    from concourse import mybir
    from concourse.masks import make_identity
    from contextlib import ExitStack

    dt = mybir.dt
    f32, bf16 = dt.float32, dt.bfloat16
    AF = mybir.ActivationFunctionType
    OP = mybir.AluOpType
    AX = mybir.AxisListType

    nc = bass.Bass()
    caps_d = nc.dram_tensor("caps", (NCW, D), bf16, kind="ExternalInput")
    imgs_d = nc.dram_tensor("imgs", (IR, D), bf16, kind="ExternalInput")
    wq_d = nc.dram_tensor("wq", (D, D), bf16, kind="ExternalInput")
    wk_d = nc.dram_tensor("wk", (D, D), bf16, kind="ExternalInput")
    wv_d = nc.dram_tensor("wv", (D, D), bf16, kind="ExternalInput")
    wo_d = nc.dram_tensor("wo", (D, D), bf16, kind="ExternalInput")
    bvec_d = nc.dram_tensor("bvec", (128, 16), f32, kind="ExternalInput")
    m01_d = nc.dram_tensor("mask01", (IR,), bf16, kind="ExternalInput")
    capm_d = nc.dram_tensor("capm2", (NCW, 2), f32, kind="ExternalInput")
    out_d = nc.dram_tensor("out", (BI_S, NCW), f32, kind="ExternalOutput")

    irows = [(0, 128), (128, 128), (256, 32)]

    with ExitStack() as ctx:
        tc = ctx.enter_context(tile.TileContext(nc))
        big = ctx.enter_context(tc.tile_pool(name="big", bufs=1))
        wrk = ctx.enter_context(tc.tile_pool(name="wrk", bufs=3))
        mvp = ctx.enter_context(tc.tile_pool(name="mvp", bufs=6))
        apl = ctx.enter_context(tc.tile_pool(name="apl", bufs=3))
        tpl = ctx.enter_context(tc.tile_pool(name="tpl", bufs=2))
        atp = ctx.enter_context(tc.tile_pool(name="atp", bufs=10))
        scr = ctx.enter_context(tc.tile_pool(name="scr", bufs=3))
        epp = ctx.enter_context(tc.tile_pool(name="epp", bufs=16))
        psb = ctx.enter_context(tc.tile_pool(name="psb", bufs=6, space="PSUM"))
        pst = ctx.enter_context(tc.tile_pool(name="pst", bufs=2, space="PSUM"))

        ident = big.tile([128, 128], bf16, tag="ident")
        make_identity(nc, ident)
        epsb = big.tile([128, 1], f32, tag="epsb")
        nc.vector.memset(epsb, float(LN_EPS))

        wsb = {}
        for nm, d_ in (("wq", wq_d), ("wk", wk_d), ("wv", wv_d), ("wo", wo_d)):
            t_ = big.tile([128, DC, D], bf16, tag=nm)
            nc.sync.dma_start(out=t_, in_=d_[:, :].rearrange("(k p) d -> p k d", p=128))
            wsb[nm] = t_
        bvec = big.tile([128, 16], f32, tag="bvec")
        nc.sync.dma_start(out=bvec, in_=bvec_d[:, :])
        mask = big.tile([128, BI_S, R], bf16, tag="mask")
        m01_ap = m01_d[:]
        m01_b = bass.AP(tensor=m01_ap.tensor, offset=m01_ap.offset,
                        ap=[[0, 128]] + list(m01_ap.ap))
        nc.gpsimd.dma_start(out=mask, in_=m01_b)
        capm = big.tile([128, NT, 2], f32, tag="capm")
        nc.sync.dma_start(out=capm, in_=capm_d[:, :].rearrange("(t p) c -> p t c", p=128))
        capsin = big.tile([128, NT, D], bf16, tag="capsin")
        nc.sync.dma_start(out=capsin,
                          in_=caps_d[:, :].rearrange("(t p) d -> p t d", p=128))
        imgsin = big.tile([128, 3, D], bf16, tag="imgsin")
        for ti, (r0, p) in enumerate([(0, 128), (128, 128), (256, 32)]):
            nc.sync.dma_start(out=imgsin[:p, ti, :], in_=imgs_d[r0:r0 + p, :])

        def layer_norm(x, p):
            """x: (p, D) tile -> bf16 row-normalized (no affine; folded)."""
            st = mvp.tile([128, 6], f32, tag="st")
            nc.vector.bn_stats(st[:p], x[:p])
            ag = mvp.tile([128, 2], f32, tag="ag")
            nc.vector.bn_aggr(ag[:p], st[:p])
            sg = mvp.tile([128, 1], f32, tag="sg")
            nc.scalar.activation(sg[:p], ag[:p, 1:2], AF.Sqrt, bias=epsb[:p])
            iv = mvp.tile([128, 1], f32, tag="iv")
            nc.vector.reciprocal(iv[:p], sg[:p])
            mi = mvp.tile([128, 1], f32, tag="mi")
            nc.vector.tensor_mul(mi[:p], ag[:p, 0:1], iv[:p])
            nmi = mvp.tile([128, 1], f32, tag="nmi")
            nc.scalar.mul(nmi[:p], mi[:p], -1.0)
            ln = wrk.tile([128, D], bf16, tag="ln")
            nc.scalar.activation(ln[:p], x[:p], AF.Identity,
                                 bias=nmi[:p], scale=iv[:p])
            return ln

        # ---------------- phase A: image side (per-core slice, 288 rows) ----
        lniT = big.tile([128, DC, IR], bf16, tag="lniT")
        for ti, (r0, p) in enumerate(irows):
            ln = layer_norm(imgsin[:, ti, :], p)
            for j in range(DC):
                pt = pst.tile([128, 128], bf16, tag="tr")
                nc.tensor.transpose(pt[:, :p], ln[:p, 128 * j:128 * j + 128],
                                    ident[:p, :p])
                nc.scalar.copy(out=lniT[:, j, r0:r0 + p], in_=pt[:, :p])

        # K^T = WkgT-proj(lniT) + bk'  (d on partitions)
        kT = big.tile([128, DC, IR], bf16, tag="kT")
        for j in range(DC):
            ps = psb.tile([128, IR], f32, tag="ps")
            for k in range(DC):
                nc.tensor.matmul(ps, lhsT=wsb["wk"][:, k, 128 * j:128 * j + 128],
                                 rhs=lniT[:, k, :], start=(k == 0), stop=(k == DC - 1))
            nc.scalar.activation(kT[:, j, :], ps, AF.Identity,
                                 bias=bvec[:, 4 + j:5 + j])

        # V natural -> row-center -> V'^T (+bvc per-partition)
        vpT = big.tile([128, DC, IR], bf16, tag="vpT")
        for (r0, p) in irows:
            ps = psb.tile([128, D], f32, tag="ps")
            for k in range(DC):
                nc.tensor.matmul(ps[:p], lhsT=lniT[:, k, r0:r0 + p],
                                 rhs=wsb["wv"][:, k, :], start=(k == 0), stop=(k == DC - 1))
            ms = mvp.tile([128, 1], f32, tag="ms")
            nc.vector.reduce_sum(ms[:p], ps[:p], axis=AX.X)
            nmu = mvp.tile([128, 1], f32, tag="nmu")
            nc.scalar.mul(nmu[:p], ms[:p], -1.0 / D)
            vb = wrk.tile([128, D], bf16, tag="vb")
            nc.scalar.activation(vb[:p], ps[:p], AF.Identity, bias=nmu[:p])
            for j in range(DC):
                pt = pst.tile([128, 128], bf16, tag="tr")
                nc.tensor.transpose(pt[:, :p], vb[:p, 128 * j:128 * j + 128],
                                    ident[:p, :p])
                nc.scalar.activation(vpT[:, j, r0:r0 + p], pt[:, :p], AF.Identity,
                                     bias=bvec[:, 12 + j:13 + j])

        # P^T = Wo_g4 @ V'^T + pc
        pT = big.tile([128, DC, IR], bf16, tag="pT")
        for j in range(DC):
            ps = psb.tile([128, IR], f32, tag="ps")
            for k in range(DC):
                nc.tensor.matmul(ps, lhsT=wsb["wo"][:, k, 128 * j:128 * j + 128],
                                 rhs=vpT[:, k, :], start=(k == 0), stop=(k == DC - 1))
            nc.scalar.activation(pT[:, j, :], ps, AF.Identity,
                                 bias=bvec[:, 8 + j:9 + j])

        # 36x36 Grams per image: S/D and M
        sm = big.tile([36, BI_S, 72], bf16, tag="sm")
        for i in range(BI_S):
            for (src, c0_, scl) in ((vpT, 0, 1.0 / D), (pT, 36, 1.0)):
                gps = pst.tile([36, 36], f32, tag="tr")
                for k in range(DC):
                    nc.tensor.matmul(gps, lhsT=src[:, k, 36 * i:36 * i + 36],
                                     rhs=src[:, k, 36 * i:36 * i + 36],
                                     start=(k == 0), stop=(k == DC - 1))
                nc.scalar.mul(sm[:, i, c0_:c0_ + 36], gps, scl)

        # ---------------- phase B: caption LN/transpose + Q^T projection ----
        lncT = big.tile([128, DC, NCW], bf16, tag="lncT")
        qT = big.tile([128, DC, NCW], bf16, tag="qT")
        for t in range(NT):
            ln = layer_norm(capsin[:, t, :], 128)
            for j in range(DC):
                pt = pst.tile([128, 128], bf16, tag="tr")
                nc.tensor.transpose(pt, ln[:, 128 * j:128 * j + 128], ident)
                nc.scalar.copy(out=lncT[:, j, 128 * t:128 * t + 128], in_=pt)
            if t % 4 == 3:
                c = t // 4
                for j in range(DC):
                    ps = psb.tile([128, 512], f32, tag="ps")
                    for k in range(DC):
                        nc.tensor.matmul(ps, lhsT=wsb["wq"][:, k, 128 * j:128 * j + 128],
                                         rhs=lncT[:, k, 512 * c:512 * c + 512],
                                         start=(k == 0), stop=(k == DC - 1))
                    nc.scalar.activation(qT[:, j, 512 * c:512 * c + 512], ps,
                                         AF.Identity, bias=bvec[:, j:j + 1])

        # ---------------- phase C: per caption-word tile ---------------------
        for t in range(NT):
            cw = slice(128 * t, 128 * t + 128)
            psK = psb.tile([128, BI_S, R], f32, tag="ps")
            for k in range(DC):
                nc.tensor.matmul(psK, lhsT=qT[:, k, cw], rhs=kT[:, k, :],
                                 start=(k == 0), stop=(k == DC - 1))
            psT = psb.tile([128, BI_S, R], f32, tag="ps")
            for k in range(DC):
                nc.tensor.matmul(psT, lhsT=qT[:, k, cw], rhs=pT[:, k, :],
                                 start=(k == 0), stop=(k == DC - 1))
            A = apl.tile([128, BI_S, R], bf16, tag="A")
            nc.scalar.activation(A, psK, AF.Exp, scale=SCALE)
            nc.vector.tensor_mul(A, A, mask)
            tsb = tpl.tile([128, BI_S, R], bf16, tag="tsb")
            nc.scalar.copy(tsb, psT)

            ats = []
            for i in range(BI_S):
                pt = pst.tile([128, 128], bf16, tag="tr")
                nc.tensor.transpose(pt[:R, :], A[:, i, :], ident)
                at = atp.tile([64, 128], bf16, tag="at")
                nc.scalar.copy(at[:R, :], pt[:R, :])
                ats.append(at)

            psB = []
            for h in range(2):
                pb = psb.tile([128, 4, 72], f32, tag="ps")
                for m in range(4):
                    i = 4 * h + m
                    nc.tensor.matmul(pb[:, m, :], lhsT=ats[i][:R, :],
                                     rhs=sm[:, i, :], start=True, stop=True)
                psB.append(pb)

            ssum = epp.tile([128, BI_S], f32, tag="essum")
            nc.vector.reduce_sum(ssum, A, axis=AX.X)
            scT = scr.tile([128, BI_S, R], bf16, tag="scT")
            nc.vector.tensor_mul(scT, A, tsb)
            eT = epp.tile([128, BI_S], f32, tag="enT")
            nc.vector.reduce_sum(eT, scT, axis=AX.X)
            scS = scr.tile([128, BI_S, R], f32, tag="scS")
            scM = scr.tile([128, BI_S, R], f32, tag="scM")
            for h in range(2):
                sl = slice(4 * h, 4 * h + 4)
                nc.vector.tensor_mul(scS[:, sl, :], A[:, sl, :], psB[h][:, :, 0:36])
                nc.vector.tensor_mul(scM[:, sl, :], A[:, sl, :], psB[h][:, :, 36:72])
            eS = epp.tile([128, BI_S], f32, tag="enS")
            nc.vector.reduce_sum(eS, scS, axis=AX.X)
            eM = epp.tile([128, BI_S], f32, tag="enM")
            nc.vector.reduce_sum(eM, scM, axis=AX.X)

            sqe = epp.tile([128, BI_S], f32, tag="esqe")
            nc.scalar.activation(sqe, ssum, AF.Square, scale=float(np.sqrt(LN_EPS)))
            t2 = epp.tile([128, BI_S], f32, tag="et2")
            nc.gpsimd.tensor_add(t2, sqe, eS)
            gg = epp.tile([128, BI_S], f32, tag="egg")
            nc.scalar.activation(gg, t2, AF.Sqrt)
            qm = epp.tile([128, BI_S], f32, tag="eqm")
            nc.scalar.activation(qm, eM, AF.Sqrt)
            ge = epp.tile([128, BI_S], f32, tag="ege")
            nc.gpsimd.tensor_scalar_mul(ge, gg, float(EPS))
            den = epp.tile([128, BI_S], f32, tag="eden")
            nc.gpsimd.tensor_add(den, ge, qm)
            rr = epp.tile([128, BI_S], f32, tag="err")
            nc.vector.reciprocal(rr, den)
            s0 = epp.tile([128, BI_S], f32, tag="es0")
            nc.gpsimd.tensor_mul(s0, eT, rr)
            sf = epp.tile([128, BI_S], f32, tag="esf")
            nc.scalar.activation(sf, s0, AF.Identity,
                                 bias=capm[:, t, 1:2], scale=capm[:, t, 0:1])
            nc.sync.dma_start(out=out_d[:, cw].rearrange("i p -> p i"), in_=sf)

    return nc


def _get_program():
    if "nc" not in _PROG_CACHE:
        _PROG_CACHE["nc"] = _build_program()
    return _PROG_CACHE["nc"]


# ------------------------------------------------------------------- driver --
def kernel(imgs, caps, img_lens, cap_lens,
           Wq, bq, Wk, bk, Wv, bv, Wo, bo,
           g1, b1, g2, b2, g3, b3, g4, b4):
    global LAST_EXEC_NS, LAST_TRACE
    args = dict(imgs=imgs, caps=caps, img_lens=img_lens, cap_lens=cap_lens,
                Wq=Wq, bq=bq, Wk=Wk, bk=bk, Wv=Wv, bv=bv, Wo=Wo, bo=bo,
                g1=g1, b1=b1, g2=g2, b2=b2, g3=g3, b3=b3, g4=g4, b4=b4)
    args = {k: np.asarray(v, np.float32) if np.asarray(v).dtype != np.int32
            else np.asarray(v) for k, v in args.items()}
    imgs, caps = args["imgs"], args["caps"]
    img_lens, cap_lens = np.asarray(img_lens, np.int32), np.asarray(cap_lens, np.int32)
    c0 = args["Wo"] @ args["b4"] + args["bo"]
    if (imgs.shape, caps.shape) != ((Bi, R, D), (Bc, W, D)) or np.abs(c0).max() != 0:
        return _np_kernel(**args)
    try:
        return _device_kernel(args, img_lens, cap_lens)
    except Exception:
        import traceback
        traceback.print_exc()
        print("kernel: device path failed; falling back to numpy", file=sys.stderr)
        return _np_kernel(**args)


def _device_kernel(a, img_lens, cap_lens):
    global LAST_EXEC_NS, LAST_TRACE
    import ml_dtypes
    from concourse.bass_utils import run_bass_kernel_spmd

    bf = ml_dtypes.bfloat16
    img_valid = (np.arange(R)[None, :] < img_lens[:, None])
    cap_valid = (np.arange(W)[None, :] < cap_lens[:, None])
    imgs_m = (a["imgs"] * img_valid[..., None]).reshape(Bi * R, D)
    caps_m = (a["caps"] * cap_valid[..., None]).reshape(NCW, D)

    WqgT = np.ascontiguousarray((a["Wq"] * a["g1"][None, :]).T).astype(bf)
    WkgT = np.ascontiguousarray((a["Wk"] * a["g2"][None, :]).T).astype(bf)
    WvgT = np.ascontiguousarray((a["Wv"] * a["g3"][None, :]).T).astype(bf)
    WogT = np.ascontiguousarray((a["Wo"] * a["g4"][None, :]).T).astype(bf)
    qb = a["Wq"] @ a["b1"] + a["bq"]
    bk_ = a["Wk"] @ a["b2"] + a["bk"]
    bv_ = a["Wv"] @ a["b3"] + a["bv"]
    bvc = (bv_ - bv_.mean()).astype(np.float32)
    pc = (a["Wo"] * a["g4"][None, :]) @ bvc
    bvec = np.stack([qb.reshape(DC, 128), bk_.reshape(DC, 128),
                     pc.reshape(DC, 128), bvc.reshape(DC, 128)],
                    axis=0).reshape(16, 128).T
    bvec = np.ascontiguousarray(bvec, dtype=np.float32)  # (128, 16)

    capm = cap_valid.reshape(NCW, 1).astype(np.float32)
    capm2 = np.ascontiguousarray(np.concatenate([capm, capm - 1.0], axis=1))

    caps_bf = np.ascontiguousarray(caps_m).astype(bf)
    in_maps = []
    for c in range(N_CORES):
        sl = slice(c * BI_S * R, (c + 1) * BI_S * R)
        in_maps.append({
            "caps": caps_bf,
            "imgs": np.ascontiguousarray(imgs_m[sl]).astype(bf),
            "wq": WqgT, "wk": WkgT, "wv": WvgT, "wo": WogT,
            "bvec": bvec,
            "mask01": np.ascontiguousarray(
                img_valid[c * BI_S:(c + 1) * BI_S].reshape(IR)).astype(bf),
            "capm2": capm2,
        })

    nc = _get_program()
    trace = bool(os.environ.get("BASS_KTRACE"))
    kw = {}
    if trace:
        kw = dict(trace=True, tmpdir=os.environ.get("BASS_KTRACE_DIR") or None)
    res = run_bass_kernel_spmd(nc, in_maps, list(range(N_CORES)), **kw)
    if trace:
        LAST_EXEC_NS = res.exec_time_ns
        LAST_TRACE = res.profile_json
    out = np.concatenate(
        [r["out"].reshape(BI_S, Bc, W) for r in res.results], axis=0)
    return np.ascontiguousarray(out.astype(np.float32))
